# revision 56
# baseline (speedup 1.0000x reference)
"""BitConv2d (ternary-weight 3x3 conv, power-of-two rescale) on 8 TRN2 NeuronCores.

Strategy (v2 — DMA-roofline oriented):
  - Data-parallel over batch: 32 images -> 4 per core (2 image pairs).
  - Host ships activations as f16 (halves input HBM traffic; quantization
    boundary flips contribute <0.5% rel err) and receives outputs as bf16
    (halves output traffic; +0.2% rel err).  All quantization/conv/rescale
    math still runs on-device.
  - Input loads are 2x [128, 56, 112] f16 per pair (1.6 MB each, all 16
    SDMA engines, 12.5 KB contiguous per partition).
  - Activation quantization: i16 = RNE(64*x + 128) on GPSIMD, then
    bf16 = clip(i16, 64, 192) on DVE -> v = x_int + 128 exactly.
    Padded border cells are memset to 128; the offset folds into the bias.
  - Conv as 9 accumulating matmuls per output tile (K=Cin=64, M=Cout=64),
    4-quadrant packing: rows 0-63 = image (2k), rows 64-127 = image (2k+1),
    cols 0-63 = even 4-row block, cols 64-127 = odd block.  Both blocks of
    one image accumulate into ONE [128,448] psum tile (1 bank), so the
    epilogue runs on all 128 lanes.  Taps are the outer loop over an
    iteration pair (16 output rows), quartering LDWEIGHTS traffic.
  - Epilogue y = psum * 2^(act_exp+s_exp[c]) + bias'[c] -> bf16, image a on
    DVE, image b on ACT, accumulated into per-image [128, 14, 448] tiles,
    stored with 2 large DMAs per image (~800 KB each).
"""

import numpy as np
import ml_dtypes
from contextlib import ExitStack

_NC_CACHE = {}

N_CORES = 8
H = W = 112
HP = H + 2  # padded
CIN = COUT = 64
P = 128
IMGS_PER_CORE = 4
ROWS_PER_CHUNK = 8            # quantization chunk (input rows)
ROWS_PER_TILE = 4             # output rows per matmul tile (N = 4*112 = 448)
NFREE = ROWS_PER_TILE * W     # 448 (PSUM bank limit: 512 f32/partition)
N_ITERS = H // (2 * ROWS_PER_TILE)   # 14 conv iterations (8 rows each)
N_IP = N_ITERS // 2                  # 7 iteration pairs
N_CHUNKS = H // ROWS_PER_CHUNK       # 14 quant chunks
HR = 58                              # rows per xq half-tile (padded)


def _patch_tile_drain(tile_mod):
    """This walrus build rejects a Drain carrying many sync waits; split the
    final Tile drain into single-wait sync nops."""
    from concourse.vector_clock import ScopedClock, VectorClock

    if getattr(tile_mod.TileContext, "_drain_patched", False):
        return

    def _drain_and_barrier_split(self, tick_clock, wait_clock):
        vclock = tick_clock.global_clock
        n = len(vclock)
        for proc in range(n):
            t = vclock[proc]
            if t <= 0:
                continue
            vec = [0] * n
            vec[proc] = t
            nop = self.nc.sync.nop()
            wait_clock.add_sem_waits(nop.ins, ScopedClock({None: VectorClock(vec)}))
        self.nc.sync.drain()
        assert self.sems is not None
        popped = self.nc._tile_sem_poison_stack.pop()
        assert popped is self._sem_poison
        self.nc.all_engine_barrier()
        self.nc.clear_and_free_semaphores(list(self.sems.allocated().values()))
        self.nc.all_engine_barrier()

    tile_mod.TileContext._drain_and_barrier = _drain_and_barrier_split
    tile_mod.TileContext._drain_patched = True


def _split_multi_syncs(nc):
    """This walrus build accepts at most ONE sync wait (and one update) per
    instruction.  Hoist extra waits onto preceding nops and extra updates onto
    following nops (same engine, so ordering semantics are preserved)."""
    import concourse.mybir as mybir

    fn = nc.m.functions[0]
    ctr = 0
    for bb in fn.blocks:
        new_insts = []
        for inst in bb.instructions:
            si = inst.sync_info
            pre, post = [], []
            if si is not None and si.on_wait and len(si.on_wait) > 1:
                for w in list(si.on_wait[:-1]):
                    ctr += 1
                    pre.append(
                        mybir.InstNoOp(
                            name=f"wsplit_nop_{ctr}",
                            engine=inst.engine,
                            sync_info=mybir.SyncInfo(on_wait=[w], on_update=[]),
                        )
                    )
                si.on_wait = [si.on_wait[-1]]
            if (
                si is not None
                and si.on_update
                and len(si.on_update) > 1
                and not isinstance(inst, (mybir.InstDMACopy, mybir.InstDMA))
            ):
                for u in list(si.on_update[1:]):
                    ctr += 1
                    post.append(
                        mybir.InstNoOp(
                            name=f"usplit_nop_{ctr}",
                            engine=inst.engine,
                            sync_info=mybir.SyncInfo(on_wait=[], on_update=[u]),
                        )
                    )
                si.on_update = [si.on_update[0]]
            new_insts.extend(pre)
            new_insts.append(inst)
            new_insts.extend(post)
        if len(new_insts) != len(bb.instructions):
            bb.instructions[:] = new_insts
    for bb in fn.blocks:
        for inst in bb.instructions:
            if inst.name.startswith(("wsplit_nop_", "usplit_nop_")):
                if inst.name not in nc.inst_map:
                    nc.register_instruction(inst)
    return ctr


def build_nc(act_exp=-6):
    import concourse.bass as bass
    import concourse.mybir as mybir
    import concourse.tile as tile

    _patch_tile_drain(tile)

    inv_step = float(2.0 ** (-act_exp))       # x_int = RNE(x * inv_step)
    lim = float(min(inv_step, 127.0))         # |x_int| clamp (clip(x,±1) + QMAX)

    f32 = mybir.dt.float32
    f16 = mybir.dt.float16
    bf16 = mybir.dt.bfloat16
    i16 = mybir.dt.int16
    Alu = mybir.AluOpType
    Act = mybir.ActivationFunctionType

    nc = bass.Bass(trn_type="TRN2")
    x4 = nc.dram_tensor("x4", (IMGS_PER_CORE * CIN, H, W), f16, kind="ExternalInput")
    wsb = nc.dram_tensor("wsb", (P, 9 * COUT), bf16, kind="ExternalInput")
    sb = nc.dram_tensor("sb", (P, 2), f32, kind="ExternalInput")
    # y4 layout = SBUF-native dump [img, parity*64+ch, iter, 4*112]; the host
    # unscrambles (row = 8*iter + 4*parity + r).  Keeps store descriptors at
    # 12.5 KB/partition instead of 896 B.
    y4 = nc.dram_tensor("y4", (IMGS_PER_CORE * P, N_ITERS * NFREE), bf16,
                        kind="ExternalOutput")

    with tile.TileContext(nc) as tc, ExitStack() as ctx:
        const_pool = ctx.enter_context(tc.tile_pool(name="const", bufs=1))
        stg_pool = ctx.enter_context(tc.tile_pool(name="stg", bufs=2))
        xq_pool = ctx.enter_context(tc.tile_pool(name="xq", bufs=2))
        rnd_pool = ctx.enter_context(tc.tile_pool(name="rnd", bufs=4))
        out_pool = ctx.enter_context(tc.tile_pool(name="out", bufs=4))
        psum_pool = ctx.enter_context(
            tc.tile_pool(name="psum", bufs=8, space=bass.MemorySpace.PSUM)
        )

        # first input chunk issued before anything else: its ~128-descriptor
        # HWDGE generation time sits on the pipeline-fill critical path
        stg0 = stg_pool.tile([P, H, W], f16, name="stg")
        nc.sync.dma_start(stg0[:, 0:16, :], x4[0:P, 0:16, :])

        w_t = const_pool.tile([P, 9 * COUT], bf16)
        nc.sync.dma_start(w_t[:], wsb[:])
        sb_t = const_pool.tile([P, 2], f32)
        nc.sync.dma_start(sb_t[:], sb[:])

        # PE warm-up: the HAM clock gate defaults to 4/8 (1.2 GHz) and only
        # reaches 8/8 after ~3.4us of sustained matmul activity.  The PE is
        # otherwise idle during the ~17us load+quant prologue; dummy matmuls
        # keep it busy so the real conv starts at full clock.
        warm = const_pool.tile([P, NFREE], bf16)
        nc.vector.memset(warm[:], 1.0)
        wps = psum_pool.tile([P, NFREE], f32, name="ps")
        for _ in range(18):
            nc.tensor.matmul(wps[:], warm[:, 0:P], warm[:], start=True,
                             stop=True)

        def load_pair(k, bounds=(0, 28, 56, 84, H), stg=None):
            # chunked so the first conv iteration (input rows 0-8) can start
            # after a fraction of the pair's load has landed
            if stg is None:
                stg = stg_pool.tile([P, H, W], f16, name="stg")
            for r0, r1 in zip(bounds[:-1], bounds[1:]):
                nc.sync.dma_start(stg[:, r0:r1, :],
                                  x4[P * k:P * k + P, r0:r1, :])
            return stg

        def alloc_xq():
            # split padded image vertically: top = padded rows 0..57,
            # bottom = padded rows 56..113 (2-row halo overlap)
            xqt = xq_pool.tile([P, HR, HP], bf16)
            xqb = xq_pool.tile([P, HR, HP], bf16)
            nc.vector.memset(xqt[:, 0, :], 128.0)
            nc.vector.memset(xqt[:, :, 0], 128.0)
            nc.vector.memset(xqt[:, :, HP - 1], 128.0)
            nc.vector.memset(xqb[:, HR - 1, :], 128.0)
            nc.vector.memset(xqb[:, :, 0], 128.0)
            nc.vector.memset(xqb[:, :, HP - 1], 128.0)
            return xqt, xqb

        def emit_quant(stg, tiles, ch):
            xqt, xqb = tiles
            r0 = ch * ROWS_PER_CHUNK
            r1 = r0 + ROWS_PER_CHUNK - 1
            rnd = rnd_pool.tile([P, ROWS_PER_CHUNK, W], i16)
            # i16 = RNE(64*x + 128): hw f16->i16 cast rounds to nearest even
            nc.gpsimd.tensor_scalar(out=rnd[:], in0=stg[:, r0:r1 + 1, :],
                                    scalar1=inv_step, scalar2=128.0,
                                    op0=Alu.mult, op1=Alu.add)
            # bf16 = clip(i16, 64, 192) == x_int + 128, into padded halves
            segs = []
            ta, tb = max(r0, 0), min(r1, 56)       # top covers img rows 0..56
            if ta <= tb:
                segs.append((xqt, ta + 1, ta - r0, tb - ta + 1))
            ba, bb = max(r0, 55), r1               # bottom covers img rows 55..112
            if ba <= bb:
                segs.append((xqb, ba - 55, ba - r0, bb - ba + 1))
            for t, dst0, src0, nrows in segs:
                nc.vector.tensor_scalar(
                    out=t[:, dst0:dst0 + nrows, 1:1 + W],
                    in0=rnd[:, src0:src0 + nrows, :],
                    scalar1=128 - lim, scalar2=128 + lim,
                    op0=Alu.max, op1=Alu.min,
                )

        def emit_conv_ip(k, tiles, its, outs, act_only=False):
            # Iteration group (1-2 iterations x 8 output rows), taps outermost
            # so each weight set loads once per image per tap (4 matmuls/load).
            # pss[img][sub]: [128,448] psum; partitions 0-63 = rows
            # 8it..8it+3 (col group 0), partitions 64-127 = rows 8it+4..+7.
            nsub = len(its)
            pss = [[psum_pool.tile([P, NFREE], f32, name="ps")
                    for _s in range(nsub)] for _i in range(2)]
            subs = []
            for it in its:
                if it < 7:
                    subs.append((tiles[0], 8 * it))
                else:
                    subs.append((tiles[1], 8 * it - 56))
            for tap in range(9):
                dh, dw = divmod(tap, 3)
                st, sp = tap == 0, tap == 8
                for ih_i, ih in enumerate((0, 64)):
                    for sub in range(nsub):
                        xq, base = subs[sub]
                        for blk in range(2):
                            hs = base + ROWS_PER_TILE * blk + dh
                            rhs = xq[ih:ih + 64, hs:hs + ROWS_PER_TILE,
                                     dw:dw + W]
                            # split M=64 into two 32-col strips: 4 col groups
                            # run concurrently and each LDWEIGHTS is half as
                            # long, cutting per-quadrant weight-load stalls
                            for ch in range(2):
                                c0 = 64 * blk + 32 * ch
                                nc.tensor.matmul(
                                    pss[ih_i][sub][c0:c0 + 32, :],
                                    w_t[ih:ih + 64,
                                        tap * 64 + 32 * ch:
                                        tap * 64 + 32 * ch + 32],
                                    rhs,
                                    start=st, stop=sp,
                                    tile_position=(ih, c0),
                                )
            for ih_i in range(2):
                for sub in range(nsub):
                    it = its[sub]
                    dst = outs[ih_i][:, it, :]
                    # engine choice: DVE for img a / ACT for img b normally;
                    # ACT-only while DVE is clogged with quant clips (pair 0)
                    if ih_i == 0 and not act_only:
                        eng = "vector"
                    else:
                        eng = "scalar"
                    if eng == "scalar":
                        nc.scalar.activation(
                            dst, pss[ih_i][sub][:], Act.Identity,
                            scale=sb_t[:, 0:1], bias=sb_t[:, 1:2],
                        )
                    else:
                        getattr(nc, eng).tensor_scalar(
                            out=dst, in0=pss[ih_i][sub][:],
                            scalar1=sb_t[:, 0:1], scalar2=sb_t[:, 1:2],
                            op0=Alu.mult, op1=Alu.add,
                        )

        def store_iters(k, outs, it0, nit):
            for img in range(2):
                dst = y4[P * (2 * k + img):P * (2 * k + img + 1),
                         NFREE * it0:NFREE * (it0 + nit)]
                nc.sync.dma_start(dst, outs[img][:, it0:it0 + nit, :])

        # ---- schedule ----
        load_pair(0, bounds=(16, 44, 72, H), stg=stg0)
        tiles0 = alloc_xq()
        for ch in range(N_CHUNKS):
            emit_quant(stg0, tiles0, ch)

        stg1 = load_pair(1)
        tiles1 = alloc_xq()
        outs0 = [out_pool.tile([P, N_ITERS, NFREE], bf16, name="ob")
                 for _i in range(2)]
        for ip in range(N_IP):
            # conv first: keeps pair-0 epilogues ahead of pair-1 quant clips
            # in the DVE queue (avoids head-of-line blocking on psum drain)
            emit_conv_ip(0, tiles0, (2 * ip, 2 * ip + 1), outs0,
                         act_only=True)
            emit_quant(stg1, tiles1, 2 * ip)
            emit_quant(stg1, tiles1, 2 * ip + 1)
            if ip == 3:
                store_iters(0, outs0, 0, 7)
            elif ip == 6:
                store_iters(0, outs0, 7, 5)
        store_iters(0, outs0, 12, 2)

        outs1 = [out_pool.tile([P, N_ITERS, NFREE], bf16, name="ob")
                 for _i in range(2)]
        # last two iterations as single-iter groups: the closing epilogue is
        # one op per engine, so the final stores launch ~0.9us earlier
        for ip in range(N_IP - 1):
            emit_conv_ip(1, tiles1, (2 * ip, 2 * ip + 1), outs1)
            if ip == 3:
                store_iters(1, outs1, 0, 7)
        emit_conv_ip(1, tiles1, (12,), outs1)
        store_iters(1, outs1, 7, 5)
        emit_conv_ip(1, tiles1, (13,), outs1)
        store_iters(1, outs1, 12, 1)
        store_iters(1, outs1, 13, 1)

    _split_multi_syncs(nc)
    nc.finalize()
    return nc


def _host_prep(w_q, s_exp, bias, act_exp):
    """Weights in lhsT layout (dup on both partition halves) + scale/bias fold."""
    w_half = np.transpose(w_q, (1, 2, 3, 0)).reshape(CIN, 9 * COUT)  # [ci, tap*64+co]
    wsb = np.concatenate([w_half, w_half], axis=0).astype(ml_dtypes.bfloat16)

    s_exp = np.asarray(s_exp).reshape(-1).astype(np.float64)
    scale = np.exp2(float(act_exp) + s_exp)                       # [64]
    wsum = w_q.astype(np.float64).sum(axis=(1, 2, 3))             # [64]
    bias_c = np.asarray(bias).astype(np.float64) - 128.0 * wsum * scale
    col_scale = np.tile(scale, 2).astype(np.float32)
    col_bias = np.tile(bias_c, 2).astype(np.float32)
    sb = np.stack([col_scale, col_bias], axis=1)                  # [128, 2] f32
    return wsb, sb


def kernel(x, w_q, s_exp, bias, act_exp):
    from concourse.bass_utils import run_bass_kernel_spmd

    xh = np.ascontiguousarray(np.asarray(x)).astype(np.float16)
    wsb, sb = _host_prep(np.asarray(w_q), s_exp, bias, int(act_exp))

    key = ("nc", int(act_exp))
    if key not in _NC_CACHE:
        _NC_CACHE[key] = build_nc(int(act_exp))
    nc = _NC_CACHE[key]

    in_maps = [
        {"x4": xh[4 * c:4 * c + 4].reshape(IMGS_PER_CORE * CIN, H, W),
         "wsb": wsb, "sb": sb}
        for c in range(N_CORES)
    ]
    res = run_bass_kernel_spmd(nc, in_maps, core_ids=list(range(N_CORES)))
    # unscramble [img, parity*64+ch, iter, r, w] -> [img, ch, 8*iter+4*parity+r, w]
    outs = []
    for c in range(N_CORES):
        a = np.asarray(res.results[c]["y4"]).reshape(
            IMGS_PER_CORE, 2, COUT, N_ITERS, ROWS_PER_TILE, W)
        outs.append(a.transpose(0, 2, 3, 1, 4, 5).reshape(
            IMGS_PER_CORE, COUT, H, W))
    return np.concatenate(outs, axis=0).astype(np.float32)


# revision 60
# speedup vs baseline: 1.3579x; 1.3579x over previous
"""BitConv2d (ternary-weight 3x3 conv, power-of-two rescale) on 8 TRN2 NeuronCores.

Strategy (v2 — DMA-roofline oriented):
  - Data-parallel over batch: 32 images -> 4 per core (2 image pairs).
  - Host ships activations as f16 (halves input HBM traffic; quantization
    boundary flips contribute <0.5% rel err) and receives outputs as bf16
    (halves output traffic; +0.2% rel err).  All quantization/conv/rescale
    math still runs on-device.
  - Input loads are 2x [128, 56, 112] f16 per pair (1.6 MB each, all 16
    SDMA engines, 12.5 KB contiguous per partition).
  - Activation quantization: i16 = RNE(64*x + 128) on GPSIMD, then
    bf16 = clip(i16, 64, 192) on DVE -> v = x_int + 128 exactly.
    Padded border cells are memset to 128; the offset folds into the bias.
  - Conv as 9 accumulating matmuls per output tile (K=Cin=64, M=Cout=64),
    4-quadrant packing: rows 0-63 = image (2k), rows 64-127 = image (2k+1),
    cols 0-63 = even 4-row block, cols 64-127 = odd block.  Both blocks of
    one image accumulate into ONE [128,448] psum tile (1 bank), so the
    epilogue runs on all 128 lanes.  Taps are the outer loop over an
    iteration pair (16 output rows), quartering LDWEIGHTS traffic.
  - Epilogue y = psum * 2^(act_exp+s_exp[c]) + bias'[c] -> bf16, image a on
    DVE, image b on ACT, accumulated into per-image [128, 14, 448] tiles,
    stored with 2 large DMAs per image (~800 KB each).
"""

import numpy as np
import ml_dtypes
from contextlib import ExitStack

_NC_CACHE = {}

N_CORES = 8
H = W = 112
HP = H + 2  # padded
CIN = COUT = 64
P = 128
IMGS_PER_CORE = 4
ROWS_PER_CHUNK = 8            # quantization chunk (input rows)
ROWS_PER_TILE = 4             # output rows per matmul tile (N = 4*112 = 448)
NFREE = ROWS_PER_TILE * W     # 448 (PSUM bank limit: 512 f32/partition)
N_ITERS = H // (2 * ROWS_PER_TILE)   # 14 conv iterations (8 rows each)
N_IP = N_ITERS // 2                  # 7 iteration pairs
N_CHUNKS = H // ROWS_PER_CHUNK       # 14 quant chunks
HR = 58                              # rows per xq half-tile (padded)


def _patch_tile_drain(tile_mod):
    """This walrus build rejects a Drain carrying many sync waits; split the
    final Tile drain into single-wait sync nops."""
    from concourse.vector_clock import ScopedClock, VectorClock

    if getattr(tile_mod.TileContext, "_drain_patched", False):
        return

    def _drain_and_barrier_split(self, tick_clock, wait_clock):
        vclock = tick_clock.global_clock
        n = len(vclock)
        for proc in range(n):
            t = vclock[proc]
            if t <= 0:
                continue
            vec = [0] * n
            vec[proc] = t
            nop = self.nc.sync.nop()
            wait_clock.add_sem_waits(nop.ins, ScopedClock({None: VectorClock(vec)}))
        self.nc.sync.drain()
        assert self.sems is not None
        popped = self.nc._tile_sem_poison_stack.pop()
        assert popped is self._sem_poison
        self.nc.all_engine_barrier()
        self.nc.clear_and_free_semaphores(list(self.sems.allocated().values()))
        self.nc.all_engine_barrier()

    tile_mod.TileContext._drain_and_barrier = _drain_and_barrier_split
    tile_mod.TileContext._drain_patched = True


def _split_multi_syncs(nc):
    """This walrus build accepts at most ONE sync wait (and one update) per
    instruction.  Hoist extra waits onto preceding nops and extra updates onto
    following nops (same engine, so ordering semantics are preserved)."""
    import concourse.mybir as mybir

    fn = nc.m.functions[0]
    ctr = 0
    for bb in fn.blocks:
        new_insts = []
        for inst in bb.instructions:
            si = inst.sync_info
            pre, post = [], []
            if si is not None and si.on_wait and len(si.on_wait) > 1:
                for w in list(si.on_wait[:-1]):
                    ctr += 1
                    pre.append(
                        mybir.InstNoOp(
                            name=f"wsplit_nop_{ctr}",
                            engine=inst.engine,
                            sync_info=mybir.SyncInfo(on_wait=[w], on_update=[]),
                        )
                    )
                si.on_wait = [si.on_wait[-1]]
            if (
                si is not None
                and si.on_update
                and len(si.on_update) > 1
                and not isinstance(inst, (mybir.InstDMACopy, mybir.InstDMA))
            ):
                for u in list(si.on_update[1:]):
                    ctr += 1
                    post.append(
                        mybir.InstNoOp(
                            name=f"usplit_nop_{ctr}",
                            engine=inst.engine,
                            sync_info=mybir.SyncInfo(on_wait=[], on_update=[u]),
                        )
                    )
                si.on_update = [si.on_update[0]]
            new_insts.extend(pre)
            new_insts.append(inst)
            new_insts.extend(post)
        if len(new_insts) != len(bb.instructions):
            bb.instructions[:] = new_insts
    for bb in fn.blocks:
        for inst in bb.instructions:
            if inst.name.startswith(("wsplit_nop_", "usplit_nop_")):
                if inst.name not in nc.inst_map:
                    nc.register_instruction(inst)
    return ctr


def build_nc(act_exp=-6):
    import concourse.bass as bass
    import concourse.mybir as mybir
    import concourse.tile as tile

    _patch_tile_drain(tile)

    inv_step = float(2.0 ** (-act_exp))       # x_int = RNE(x * inv_step)
    lim = float(min(inv_step, 127.0))         # |x_int| clamp (clip(x,±1) + QMAX)

    f32 = mybir.dt.float32
    f16 = mybir.dt.float16
    bf16 = mybir.dt.bfloat16
    i16 = mybir.dt.int16
    Alu = mybir.AluOpType
    Act = mybir.ActivationFunctionType

    nc = bass.Bass(trn_type="TRN2")
    x4 = nc.dram_tensor("x4", (IMGS_PER_CORE * CIN, H, W), f16, kind="ExternalInput")
    wsb = nc.dram_tensor("wsb", (P, 9 * COUT), bf16, kind="ExternalInput")
    sb = nc.dram_tensor("sb", (P, 2), f32, kind="ExternalInput")
    # y4 layout = SBUF-native dump [img, parity*64+ch, iter, 4*112]; the host
    # unscrambles (row = 8*iter + 4*parity + r).  Keeps store descriptors at
    # 12.5 KB/partition instead of 896 B.
    y4 = nc.dram_tensor("y4", (IMGS_PER_CORE * P, N_ITERS * NFREE), bf16,
                        kind="ExternalOutput")

    with tile.TileContext(nc) as tc, ExitStack() as ctx:
        const_pool = ctx.enter_context(tc.tile_pool(name="const", bufs=1))
        stg_pool = ctx.enter_context(tc.tile_pool(name="stg", bufs=2))
        xq_pool = ctx.enter_context(tc.tile_pool(name="xq", bufs=2))
        rnd_pool = ctx.enter_context(tc.tile_pool(name="rnd", bufs=4))
        out_pool = ctx.enter_context(tc.tile_pool(name="out", bufs=4))
        psum_pool = ctx.enter_context(
            tc.tile_pool(name="psum", bufs=8, space=bass.MemorySpace.PSUM)
        )

        # first input chunk issued before anything else: its ~128-descriptor
        # HWDGE generation time sits on the pipeline-fill critical path
        stg0 = stg_pool.tile([P, H, W], f16, name="stg")
        nc.sync.dma_start(stg0[:, 0:16, :], x4[0:P, 0:16, :])

        w_t = const_pool.tile([P, 9 * COUT], bf16)
        nc.sync.dma_start(w_t[:], wsb[:])
        sb_t = const_pool.tile([P, 2], f32)
        nc.sync.dma_start(sb_t[:], sb[:])

        # PE warm-up: the HAM clock gate defaults to 4/8 (1.2 GHz) and only
        # reaches 8/8 after ~3.4us of sustained matmul activity.  The PE is
        # otherwise idle during the ~17us load+quant prologue; dummy matmuls
        # keep it busy so the real conv starts at full clock.
        warm = const_pool.tile([P, NFREE], bf16)
        nc.vector.memset(warm[:], 1.0)
        wps = psum_pool.tile([P, NFREE], f32, name="ps")
        for _ in range(18):
            nc.tensor.matmul(wps[:], warm[:, 0:P], warm[:], start=True,
                             stop=True)

        def load_pair(k, bounds=(0, 28, 56, 84, H), stg=None):
            # chunked so the first conv iteration (input rows 0-8) can start
            # after a fraction of the pair's load has landed
            if stg is None:
                stg = stg_pool.tile([P, H, W], f16, name="stg")
            for r0, r1 in zip(bounds[:-1], bounds[1:]):
                nc.sync.dma_start(stg[:, r0:r1, :],
                                  x4[P * k:P * k + P, r0:r1, :])
            return stg

        def alloc_xq():
            # split padded image vertically: top = padded rows 0..57,
            # bottom = padded rows 56..113 (2-row halo overlap)
            xqt = xq_pool.tile([P, HR, HP], bf16)
            xqb = xq_pool.tile([P, HR, HP], bf16)
            nc.vector.memset(xqt[:, 0, :], 128.0)
            nc.vector.memset(xqt[:, :, 0], 128.0)
            nc.vector.memset(xqt[:, :, HP - 1], 128.0)
            nc.vector.memset(xqb[:, HR - 1, :], 128.0)
            nc.vector.memset(xqb[:, :, 0], 128.0)
            nc.vector.memset(xqb[:, :, HP - 1], 128.0)
            return xqt, xqb

        def emit_quant(stg, tiles, ch):
            xqt, xqb = tiles
            r0 = ch * ROWS_PER_CHUNK
            r1 = r0 + ROWS_PER_CHUNK - 1
            rnd = rnd_pool.tile([P, ROWS_PER_CHUNK, W], i16)
            # i16 = RNE(64*x + 128): hw f16->i16 cast rounds to nearest even
            nc.gpsimd.tensor_scalar(out=rnd[:], in0=stg[:, r0:r1 + 1, :],
                                    scalar1=inv_step, scalar2=128.0,
                                    op0=Alu.mult, op1=Alu.add)
            # bf16 = clip(i16, 64, 192) == x_int + 128, into padded halves
            segs = []
            ta, tb = max(r0, 0), min(r1, 56)       # top covers img rows 0..56
            if ta <= tb:
                segs.append((xqt, ta + 1, ta - r0, tb - ta + 1))
            ba, bb = max(r0, 55), r1               # bottom covers img rows 55..112
            if ba <= bb:
                segs.append((xqb, ba - 55, ba - r0, bb - ba + 1))
            for t, dst0, src0, nrows in segs:
                nc.vector.tensor_scalar(
                    out=t[:, dst0:dst0 + nrows, 1:1 + W],
                    in0=rnd[:, src0:src0 + nrows, :],
                    scalar1=128 - lim, scalar2=128 + lim,
                    op0=Alu.max, op1=Alu.min,
                )

        def emit_conv_ip(k, tiles, its, outs, act_only=False):
            # Iteration group (1-2 iterations x 8 output rows), taps outermost
            # so each weight set loads once per image per tap (4 matmuls/load).
            # pss[img][sub]: [128,448] psum; partitions 0-63 = rows
            # 8it..8it+3 (col group 0), partitions 64-127 = rows 8it+4..+7.
            nsub = len(its)
            pss = [[psum_pool.tile([P, NFREE], f32, name="ps")
                    for _s in range(nsub)] for _i in range(2)]
            subs = []
            for it in its:
                if it < 7:
                    subs.append((tiles[0], 8 * it))
                else:
                    subs.append((tiles[1], 8 * it - 56))
            for tap in range(9):
                dh, dw = divmod(tap, 3)
                st, sp = tap == 0, tap == 8
                for ih_i, ih in enumerate((0, 64)):
                    wt = w_t[ih:ih + 64, tap * 64:(tap + 1) * 64]
                    for sub in range(nsub):
                        xq, base = subs[sub]
                        for blk in range(2):
                            hs = base + ROWS_PER_TILE * blk + dh
                            nc.tensor.matmul(
                                pss[ih_i][sub][64 * blk:64 * blk + 64, :],
                                wt,
                                xq[ih:ih + 64, hs:hs + ROWS_PER_TILE,
                                   dw:dw + W],
                                start=st, stop=sp,
                            )
            for ih_i in range(2):
                for sub in range(nsub):
                    it = its[sub]
                    dst = outs[ih_i][:, it, :]
                    # engine choice: DVE for img a / ACT for img b normally;
                    # ACT-only while DVE is clogged with quant clips (pair 0)
                    if ih_i == 0 and not act_only:
                        eng = "vector"
                    else:
                        eng = "scalar"
                    if eng == "scalar":
                        nc.scalar.activation(
                            dst, pss[ih_i][sub][:], Act.Identity,
                            scale=sb_t[:, 0:1], bias=sb_t[:, 1:2],
                        )
                    else:
                        getattr(nc, eng).tensor_scalar(
                            out=dst, in0=pss[ih_i][sub][:],
                            scalar1=sb_t[:, 0:1], scalar2=sb_t[:, 1:2],
                            op0=Alu.mult, op1=Alu.add,
                        )

        def store_iters(k, outs, it0, nit):
            for img in range(2):
                dst = y4[P * (2 * k + img):P * (2 * k + img + 1),
                         NFREE * it0:NFREE * (it0 + nit)]
                nc.sync.dma_start(dst, outs[img][:, it0:it0 + nit, :])

        # ---- schedule ----
        load_pair(0, bounds=(16, 44, 72, H), stg=stg0)
        tiles0 = alloc_xq()
        for ch in range(N_CHUNKS):
            emit_quant(stg0, tiles0, ch)

        stg1 = load_pair(1)
        tiles1 = alloc_xq()
        outs0 = [out_pool.tile([P, N_ITERS, NFREE], bf16, name="ob")
                 for _i in range(2)]
        for ip in range(N_IP):
            # conv first: keeps pair-0 epilogues ahead of pair-1 quant clips
            # in the DVE queue (avoids head-of-line blocking on psum drain)
            emit_conv_ip(0, tiles0, (2 * ip, 2 * ip + 1), outs0,
                         act_only=True)
            emit_quant(stg1, tiles1, 2 * ip)
            emit_quant(stg1, tiles1, 2 * ip + 1)
            if ip == 3:
                store_iters(0, outs0, 0, 7)
            elif ip == 6:
                store_iters(0, outs0, 7, 5)
        store_iters(0, outs0, 12, 2)

        outs1 = [out_pool.tile([P, N_ITERS, NFREE], bf16, name="ob")
                 for _i in range(2)]
        # last two iterations as single-iter groups: the closing epilogue is
        # one op per engine, so the final stores launch ~0.9us earlier
        for ip in range(N_IP - 1):
            emit_conv_ip(1, tiles1, (2 * ip, 2 * ip + 1), outs1)
            if ip == 3:
                store_iters(1, outs1, 0, 7)
        emit_conv_ip(1, tiles1, (12,), outs1)
        store_iters(1, outs1, 7, 5)
        emit_conv_ip(1, tiles1, (13,), outs1)
        store_iters(1, outs1, 12, 1)
        store_iters(1, outs1, 13, 1)

    _split_multi_syncs(nc)
    nc.finalize()
    return nc


def _host_prep(w_q, s_exp, bias, act_exp):
    """Weights in lhsT layout (dup on both partition halves) + scale/bias fold."""
    w_half = np.transpose(w_q, (1, 2, 3, 0)).reshape(CIN, 9 * COUT)  # [ci, tap*64+co]
    wsb = np.concatenate([w_half, w_half], axis=0).astype(ml_dtypes.bfloat16)

    s_exp = np.asarray(s_exp).reshape(-1).astype(np.float64)
    scale = np.exp2(float(act_exp) + s_exp)                       # [64]
    wsum = w_q.astype(np.float64).sum(axis=(1, 2, 3))             # [64]
    bias_c = np.asarray(bias).astype(np.float64) - 128.0 * wsum * scale
    col_scale = np.tile(scale, 2).astype(np.float32)
    col_bias = np.tile(bias_c, 2).astype(np.float32)
    sb = np.stack([col_scale, col_bias], axis=1)                  # [128, 2] f32
    return wsb, sb


def kernel(x, w_q, s_exp, bias, act_exp):
    from concourse.bass_utils import run_bass_kernel_spmd

    xh = np.ascontiguousarray(np.asarray(x)).astype(np.float16)
    wsb, sb = _host_prep(np.asarray(w_q), s_exp, bias, int(act_exp))

    key = ("nc", int(act_exp))
    if key not in _NC_CACHE:
        _NC_CACHE[key] = build_nc(int(act_exp))
    nc = _NC_CACHE[key]

    in_maps = [
        {"x4": xh[4 * c:4 * c + 4].reshape(IMGS_PER_CORE * CIN, H, W),
         "wsb": wsb, "sb": sb}
        for c in range(N_CORES)
    ]
    res = run_bass_kernel_spmd(nc, in_maps, core_ids=list(range(N_CORES)))
    # unscramble [img, parity*64+ch, iter, r, w] -> [img, ch, 8*iter+4*parity+r, w]
    outs = []
    for c in range(N_CORES):
        a = np.asarray(res.results[c]["y4"]).reshape(
            IMGS_PER_CORE, 2, COUT, N_ITERS, ROWS_PER_TILE, W)
        outs.append(a.transpose(0, 2, 3, 1, 4, 5).reshape(
            IMGS_PER_CORE, COUT, H, W))
    return np.concatenate(outs, axis=0).astype(np.float32)
